# revision 33
# baseline (speedup 1.0000x reference)
"""Multi-head attention (B=2, T=2048, C=1024, H=16) on 8 trn2 cores.

Sharding: core c -> batch b = c//4, head-group g = c%4 (4 heads, proj cols
[g*256, (g+1)*256)).  Host pre-tiles all tensors into the exact SBUF layout
so every DMA is a fat contiguous 2D descriptor.  Each core computes a
partial output O_g @ Wo[g-rows] [2048, 1024] (bf16); the host sums the 4
partials per batch and adds bo.

Structure: the attention inner loop is ACT(exp)-bound (~1.1us per k-tile).
The schedule is a slot pipeline: the scores+exp stream runs one full
attention unit (16 k-tiles) ahead of the PV+norm stream, so PV matmuls
never wait on exp and only one unit's PV accumulators occupy PSUM at a
time.  QKV projection chains and out-proj (d) units are woven into slots
to fill PE slack under the ACT-bound steady state.  Normalization uses a
DVE partition-base remap (PSUM partition 64 -> SBUF partition 0) plus a
GPSIMD partition_broadcast; the out-projection contracts over 128
partitions by packing head pairs via remapped DVE writes.
"""

from collections import deque

import ml_dtypes
import numpy as np


import concourse.bass as bass
import concourse.tile as tile
from concourse import bacc, mybir
from concourse.bass_utils import run_bass_kernel_spmd

B, T, C, H, D = 2, 2048, 1024, 16, 64
N_CORES = 8
GROUPS = 4          # head-groups (cores per batch)
HG = H // GROUPS    # heads per core = 4
CG = HG * D         # proj cols per core = 256
KT = C // 128       # contraction k-tiles = 8
SCALE = D ** -0.5   # 1/8
MT = CG // 128      # stationary tiles per projection = 2
MC = 512            # chunk width (tokens)
NMC = T // MC       # 4 chunks
TT = T // 128       # 16 t-tiles
QC = 512            # q-chunk width in attention
NQC = T // QC       # 4 q-chunks
VS = D + 1          # 65: V cols + ones col per head

F32 = mybir.dt.float32
BF16 = mybir.dt.bfloat16
AF = mybir.ActivationFunctionType


def build_mha_program():
    """Build the SPMD Bass program (identical on all 8 cores)."""
    nc = bacc.Bacc("TRN2", target_bir_lowering=False, debug=False,
                   num_devices=N_CORES)

    xq = nc.dram_tensor("xq", (NMC, 128, KT, MC), BF16, kind="ExternalInput").ap()
    xk = nc.dram_tensor("xk", (NMC, 128, KT, MC), BF16, kind="ExternalInput").ap()
    xv = nc.dram_tensor("xv", (NMC, 128, KT, MC), BF16, kind="ExternalInput").ap()
    wq = nc.dram_tensor("wq", (128, KT, CG), BF16, kind="ExternalInput").ap()
    wk = nc.dram_tensor("wk", (128, KT, CG), BF16, kind="ExternalInput").ap()
    wv = nc.dram_tensor("wv", (128, KT, CG), BF16, kind="ExternalInput").ap()
    bq = nc.dram_tensor("bq", (128, MT), F32, kind="ExternalInput").ap()
    bk = nc.dram_tensor("bk", (128, MT), F32, kind="ExternalInput").ap()
    bv = nc.dram_tensor("bv", (CG,), F32, kind="ExternalInput").ap()
    wo = nc.dram_tensor("wo", (128, MT, C), BF16, kind="ExternalInput").ap()
    yp = nc.dram_tensor("yp", (T, C), BF16, kind="ExternalOutput").ap()

    with tile.TileContext(nc) as tc:
        _emit(tc, xq, xk, xv, wq, wk, wv, bq, bk, bv, wo, yp)
    nc.compile()
    return nc


def _emit(tc, xq, xk, xv, wq, wk, wv, bq, bk, bv, wo, yp):
    nc = tc.nc

    from contextlib import ExitStack
    with ExitStack() as ctx:
        consts = ctx.enter_context(tc.tile_pool(name="consts", bufs=1))
        xs_pool = ctx.enter_context(tc.tile_pool(name="xs", bufs=20))
        big = ctx.enter_context(tc.tile_pool(name="big", bufs=1))
        e_pool = ctx.enter_context(tc.tile_pool(name="e", bufs=18))
        ev_pool = ctx.enter_context(tc.tile_pool(name="ev", bufs=3))
        nrm_pool = ctx.enter_context(tc.tile_pool(name="nrm", bufs=4))
        pp = ctx.enter_context(tc.tile_pool(name="pp", bufs=2, space="PSUM"))
        pv_ps = ctx.enter_context(tc.tile_pool(name="pvps", bufs=2, space="PSUM"))
        sa_ps = ctx.enter_context(tc.tile_pool(name="saps", bufs=1, space="PSUM"))
        sb_ps = ctx.enter_context(tc.tile_pool(name="sbps", bufs=1, space="PSUM"))

        qTc = [big.tile([128, MT, MC], BF16, name=f"qTc{i}", tag=f"qTc{i}")
               for i in range(NMC)]
        kTc = [big.tile([128, MT, MC], BF16, name=f"kTc{i}", tag=f"kTc{i}")
               for i in range(NMC)]
        vc = [big.tile([128, MC // 128, HG * VS], BF16, name=f"vc{i}",
                       tag=f"vc{i}") for i in range(NMC)]
        # O packed head-pair-major: partitions 0-63 head 2hp, 64-127 head
        # 2hp+1, so the out-projection contracts over all 128 partitions.
        oc = [big.tile([128, MT, QC], BF16, name=f"oc{i}", tag=f"oc{i}")
              for i in range(NQC)]

        wq_s = consts.tile([128, KT, CG], BF16, tag="wq")
        wk_s = consts.tile([128, KT, CG], BF16, tag="wk")
        wv_s = consts.tile([128, KT, CG], BF16, tag="wv")
        wo_s = consts.tile([128, MT, C], BF16, tag="wo")
        bq_s = consts.tile([128, MT, 1], F32, tag="bq")
        bk_s = consts.tile([128, MT, 1], F32, tag="bk")
        bv_bc = consts.tile([128, CG], F32, tag="bv")

        def load_x(src, name, dtype=BF16):
            """Load one x chunk as two half tiles (kt 0-3, 4-7) so the
            first projection matmuls can start after half the transfer."""
            halves = []
            for h in range(2):
                x_t = xs_pool.tile([128, KT // 2, MC], dtype, tag="xs",
                                   name=f"{name}_{h}")
                nc.sync.dma_start(out=x_t, in_=src[:, bass.ts(h, KT // 2), :])
                halves.append(x_t)
            return halves

        def emit_qk_proj(x_h, w_s, b_s, dst, mts=(0, 1)):
            for mt in mts:
                ps = pp.tile([128, 512], F32, tag="pp")
                for kt in range(KT):
                    nc.tensor.matmul(
                        ps[:, :MC],
                        w_s[:, kt, bass.ts(mt, 128)],
                        x_h[kt // 4][:, kt % 4, :],
                        start=(kt == 0), stop=(kt == KT - 1))
                nc.vector.tensor_scalar_add(
                    dst[:, mt, :], ps[:, :MC], b_s[:, mt, :])

        def emit_v_proj(x_h, mc, subs=(0, 1, 2, 3)):
            v4 = vc[mc].rearrange("p t (h c) -> p t h c", h=HG)
            if 0 in subs:
                nc.vector.memset(v4[:, :, :, D:VS], 1.0)
            for sub in subs:
                ps = pp.tile([128, 512], F32, tag="pp")
                for kt in range(KT):
                    nc.tensor.matmul(
                        ps[:, :CG],
                        x_h[kt // 4][:, kt % 4, bass.ts(sub, 128)],
                        wv_s[:, kt, :],
                        start=(kt == 0), stop=(kt == KT - 1))
                nc.vector.tensor_add(
                    v4[:, sub, :, 0:D],
                    ps[:, :CG].rearrange("p (h c) -> p h c", h=HG),
                    bv_bc.rearrange("p (h c) -> p h c", h=HG))

        def emit_d_unit(qc, tl, cc):
            trows_out = bass.ts(qc * (QC // 128) + tl, 128)
            ps = pp.tile([128, 512], F32, tag="pp")
            for hp in range(MT):
                nc.tensor.matmul(
                    ps,
                    oc[qc][:, hp, bass.ts(tl, 128)],
                    wo_s[:, hp, bass.ts(cc, 512)],
                    start=(hp == 0), stop=(hp == MT - 1))
            ev = ev_pool.tile([128, 512], BF16, tag="ev")
            # In the post-ACT tail, split the PSUM evacuations between the
            # (now idle) ScalarE and the DVE so they don't serialize.
            if qc == NQC - 1 and (2 * tl + cc) % 2 == 0:
                nc.scalar.copy(ev, ps)
            else:
                nc.vector.tensor_copy(ev, ps)
            nc.sync.dma_start(out=yp[trows_out, bass.ts(cc, 512)], in_=ev)

        e_store = {}

        def unit_scores(qc, hp):
            """Scores + exp for one (q-chunk, head-pair): 4 blocks of 4 tk."""
            lst = e_store[(qc, hp)] = []
            for tk in range(TT):
                pool = sa_ps if tk % 2 == 0 else sb_ps
                ps = pool.tile([128, 2, 512], F32,
                               tag="sa" if tk % 2 == 0 else "sb")
                for h01 in range(2):
                    pb = h01 * D
                    nc.tensor.matmul(
                        ps[:, h01, :],
                        kTc[tk // 4][pb:pb + D, hp, bass.ts(tk % 4, 128)],
                        qTc[qc][pb:pb + D, hp, :],
                        start=True, stop=True)
                e_t = e_pool.tile([128, 2, 512], BF16, tag="e")
                nc.scalar.activation(e_t, ps, AF.Exp, scale=SCALE)
                lst.append(e_t)
                if tk % 4 == 3:
                    yield

        def unit_pv(qc, hp):
            """PV accumulation + normalization, trailing unit_scores by one
            full unit so every exp it reads is long finished."""
            lst_get = lambda tk: e_store[(qc, hp)][tk]
            po = [pv_ps.tile([128, 512], F32, tag="pv",
                             name=f"po{qc}_{hp}_{i}") for i in range(2)]
            for tk in range(TT):
                for h01 in range(2):
                    nc.tensor.matmul(
                        po[h01][0:VS, :],
                        vc[tk // 4][:, tk % 4,
                                    bass.ds((2 * hp + h01) * VS, VS)],
                        lst_get(tk)[:, h01, :],
                        start=(tk == 0), stop=(tk == TT - 1))
                if tk % 4 == 3 and tk != TT - 1:
                    yield
            # Evacuate po (O cols + den col) to SBUF right away so the two
            # PSUM banks free in ~1.4us for the next unit's PV; the actual
            # normalization then runs off the critical path.
            ou = []
            last = (qc, hp) == (NQC - 1, MT - 1)
            for h01 in range(2):
                t = nrm_pool.tile([128, 512], BF16, tag="ou")
                if last and h01 == 0:
                    nc.scalar.copy(t[0:VS, :], po[h01][0:VS, :])
                else:
                    nc.vector.tensor_copy(t[0:VS, :], po[h01][0:VS, :])
                ou.append(t)
            for h01 in range(2):
                rb = pp.tile([128, 512], F32, tag="pp")
                nc.tensor.matmul(rb[0:64, :], ones_b[D:D + 1, :],
                                 ou[h01][D:D + 1, :], start=True, stop=True)
                rec = nrm_pool.tile([64, 512], F32, tag="rec")
                nc.vector.reciprocal_approx_fast(rec, rb[0:64, :])
                nc.vector.tensor_mul(
                    oc[qc][bass.ts(h01, 64), hp, :],
                    ou[h01][0:D, :], rec)
            del e_store[(qc, hp)]
            yield

        # ---- DMA issue (priority order) -------------------------------
        nc.sync.dma_start(out=wq_s, in_=wq)
        nc.sync.dma_start(out=bq_s, in_=bq.unsqueeze(2))
        xq_t = [None] * NMC
        xk_t = [None] * NMC
        xv_t = [None] * NMC
        xq_t[0] = load_x(xq[0], "xq0")
        nc.sync.dma_start(out=wk_s, in_=wk)
        nc.sync.dma_start(out=bk_s, in_=bk.unsqueeze(2))
        xk_t[0] = load_x(xk[0], "xk0")
        nc.sync.dma_start(out=wv_s, in_=wv)
        nc.sync.dma_start(
            out=bv_bc,
            in_=bass.AP(tensor=bv.tensor, offset=bv.offset,
                        ap=[[0, 128]] + list(bv.ap)))
        xv_t[0] = load_x(xv[0], "xv0")
        for mc in range(1, NMC):
            xk_t[mc] = load_x(xk[mc], f"xk{mc}")
        for mc in range(1, NMC):
            xv_t[mc] = load_x(xv[mc], f"xv{mc}")
        xq_t[1] = load_x(xq[1], "xq1")
        nc.sync.dma_start(out=wo_s, in_=wo)

        # touch Exp once so the ACT table loads during the DMA-bound head
        warm = consts.tile([128, 1], F32, tag="warm")
        nc.vector.memset(warm[0:1, :], 0.0)
        nc.scalar.activation(warm[0:1, :], warm[0:1, :], AF.Exp)
        ones_b = consts.tile([128, 64], BF16, tag="onesb")
        nc.vector.memset(ones_b[D:D + 1, :], 1.0)

        # ---- slot pipeline --------------------------------------------
        # Each slot = one 4-tk block of the scores+exp stream (~4.4us of
        # ACT).  Fills (projection chains, out-proj units, late DMA
        # issues) are emitted AFTER the slot's scores so the exp stream
        # stays at the head of the PE queue; a chain consumed by the
        # scores of slot j is emitted in a slot < j (PE is strict FIFO —
        # emitting it later would deadlock).
        emit_qk_proj(xq_t[0], wq_s, bq_s, qTc[0])
        emit_qk_proj(xk_t[0], wk_s, bk_s, kTc[0])

        def load_late(mc):
            def f():
                xq_t[mc] = load_x(xq[mc], f"xq{mc}")
            return f

        def qk(xl, w, b, dst, mts):
            return lambda: emit_qk_proj(xl, w, b, dst, mts=mts)

        def vp(mc, subs):
            return lambda: emit_v_proj(xv_t[mc], mc, subs=subs)

        def qp(mc, mts):
            return lambda: emit_qk_proj(xq_t[mc], wq_s, bq_s, qTc[mc],
                                        mts=mts)

        # Fills at half-chain granularity, spread so no slot's PE load
        # exceeds the ~4.4us of ACT time per slot (deadlines in comments:
        # scores blk j needs K_j by slot j-1; pv(0) blk j needs V_j by
        # slot j+3; unit (qc,hp) needs Q_qc by slot 4*(2qc+hp)-1).
        forced = {
            0: [qk(xk_t[1], wk_s, bk_s, kTc[1], (0, 1))],
            1: [qk(xk_t[2], wk_s, bk_s, kTc[2], (0, 1))],
            2: [qk(xk_t[3], wk_s, bk_s, kTc[3], (0, 1)), vp(0, (0, 1))],
            3: [vp(0, (2, 3)), vp(1, (0, 1)), load_late(2)],
            4: [vp(1, (2, 3)), vp(2, (0, 1)), load_late(3)],
            5: [vp(2, (2, 3)), vp(3, (0, 1))],
            6: [vp(3, (2, 3)), qp(1, (0,))],
            7: [qp(1, (1,))],
            8: [qp(2, (0,))],
            9: [qp(2, (1,))],
            10: [qp(3, (0,))],
            11: [qp(3, (1,))],
        }
        fills = deque()

        units = [(qc, hp) for qc in range(NQC) for hp in range(MT)]
        sg = [unit_scores(*u) for u in units]
        pg = [unit_pv(*u) for u in units]
        d_queue = []
        nslots = 4 * len(units)
        # pv stream trails scores by 4 slots at the head (relaxes the K/V
        # chain deadlines there) and catches up mid-stream (double-advance
        # slots, placed so both advances stay inside one unit) to reach
        # lag 1 at the end, minimizing the post-ACT tail.
        CATCHUP = {14, 19, 24}
        pv_pos = 0
        for slot in range(nslots + 1):
            if slot < nslots:
                next(sg[slot // 4], None)
            for f in forced.pop(slot, ()):
                f()
            if slot >= 8:
                for _ in range(2):
                    if d_queue:
                        emit_d_unit(*d_queue.pop(0))
                    elif fills:
                        fills.popleft()()
                        break
            n_adv = 0 if slot < 4 else (2 if slot in CATCHUP else 1)
            for _ in range(n_adv):
                if pv_pos < nslots:
                    next(pg[pv_pos // 4], None)
                    if pv_pos % 4 == 3:
                        qc, hp = units[pv_pos // 4]
                        if hp == 1:
                            d_queue += [(qc, tl, cc)
                                        for tl in range(4) for cc in range(2)]
                    pv_pos += 1
        for u in d_queue:
            emit_d_unit(*u)


_NC_CACHE = None


def _get_program():
    global _NC_CACHE
    if _NC_CACHE is None:
        _NC_CACHE = build_mha_program()
    return _NC_CACHE


def make_in_maps(query, key, value, Wq, bq, Wk, bk, Wv, bv, Wo):
    q = np.asarray(query, np.float32).reshape(B, T, C)
    k = np.asarray(key, np.float32).reshape(B, T, C)
    v = np.asarray(value, np.float32).reshape(B, T, C)
    # [mc][p][kt][m] = x[b][mc*512 + m][kt*128 + p] : exact SBUF tile layout,
    # contiguous per chunk so each chunk DMA is one fat 2D descriptor.
    xT = {n: [np.ascontiguousarray(
                  a[b].T.reshape(KT, 128, NMC, MC).transpose(2, 1, 0, 3))
              .astype(ml_dtypes.bfloat16)
              for b in range(B)]
          for n, a in (("q", q), ("k", k), ("v", v))}
    in_maps = []
    for c in range(N_CORES):
        b, g = divmod(c, GROUPS)
        sl = slice(g * CG, (g + 1) * CG)
        wqg = np.asarray(Wq, np.float32)[:, sl].reshape(KT, 128, CG)
        wkg = np.asarray(Wk, np.float32)[:, sl].reshape(KT, 128, CG)
        wvg = np.asarray(Wv, np.float32)[:, sl].reshape(KT, 128, CG)
        wog = np.asarray(Wo, np.float32)[sl, :].reshape(MT, 128, C)
        in_maps.append({
            "xq": xT["q"][b], "xk": xT["k"][b], "xv": xT["v"][b],
            "wq": np.ascontiguousarray(wqg.transpose(1, 0, 2)).astype(ml_dtypes.bfloat16),
            "wk": np.ascontiguousarray(wkg.transpose(1, 0, 2)).astype(ml_dtypes.bfloat16),
            "wv": np.ascontiguousarray(wvg.transpose(1, 0, 2)).astype(ml_dtypes.bfloat16),
            "bq": np.ascontiguousarray(
                np.asarray(bq, np.float32)[sl].reshape(MT, 128).T),
            "bk": np.ascontiguousarray(
                np.asarray(bk, np.float32)[sl].reshape(MT, 128).T),
            "bv": np.ascontiguousarray(np.asarray(bv, np.float32)[sl]),
            "wo": np.ascontiguousarray(wog.transpose(1, 0, 2)).astype(ml_dtypes.bfloat16),
        })
    return in_maps


def assemble_output(results, bo):
    y = np.zeros((B, T, C), np.float32)
    for c, res in enumerate(results):
        y[c // GROUPS] += np.asarray(res["yp"], np.float32)
    y += np.asarray(bo, np.float32)
    return y


def kernel(query, key, value, Wq, bq, Wk, bk, Wv, bv, Wo, bo):
    nc = _get_program()
    in_maps = make_in_maps(query, key, value, Wq, bq, Wk, bk, Wv, bv, Wo)
    res = run_bass_kernel_spmd(nc, in_maps, list(range(N_CORES)))
    return assemble_output(res.results, bo)


# revision 38
# speedup vs baseline: 1.0050x; 1.0050x over previous
"""Multi-head attention (B=2, T=2048, C=1024, H=16) on 8 trn2 cores.

Sharding: core c -> batch b = c//4, head-group g = c%4 (4 heads, proj cols
[g*256, (g+1)*256)).  Host pre-tiles all tensors into the exact SBUF layout
so every DMA is a fat contiguous 2D descriptor.  Each core computes a
partial output O_g @ Wo[g-rows] [2048, 1024] (bf16); the host sums the 4
partials per batch and adds bo.

Structure: the attention inner loop is ACT(exp)-bound (~1.1us per k-tile).
The schedule is a slot pipeline: the scores+exp stream runs one full
attention unit (16 k-tiles) ahead of the PV+norm stream, so PV matmuls
never wait on exp and only one unit's PV accumulators occupy PSUM at a
time.  QKV projection chains and out-proj (d) units are woven into slots
to fill PE slack under the ACT-bound steady state.  Normalization uses a
DVE partition-base remap (PSUM partition 64 -> SBUF partition 0) plus a
GPSIMD partition_broadcast; the out-projection contracts over 128
partitions by packing head pairs via remapped DVE writes.
"""

from collections import deque

import ml_dtypes
import numpy as np


import concourse.bass as bass
import concourse.tile as tile
from concourse import bacc, mybir
from concourse.bass_utils import run_bass_kernel_spmd

B, T, C, H, D = 2, 2048, 1024, 16, 64
N_CORES = 8
GROUPS = 4          # head-groups (cores per batch)
HG = H // GROUPS    # heads per core = 4
CG = HG * D         # proj cols per core = 256
KT = C // 128       # contraction k-tiles = 8
SCALE = D ** -0.5   # 1/8
MT = CG // 128      # stationary tiles per projection = 2
MC = 512            # chunk width (tokens)
NMC = T // MC       # 4 chunks
TT = T // 128       # 16 t-tiles
QC = 512            # q-chunk width in attention
NQC = T // QC       # 4 q-chunks
VS = D + 1          # 65: V cols + ones col per head

F32 = mybir.dt.float32
BF16 = mybir.dt.bfloat16
AF = mybir.ActivationFunctionType


def build_mha_program():
    """Build the SPMD Bass program (identical on all 8 cores)."""
    nc = bacc.Bacc("TRN2", target_bir_lowering=False, debug=False,
                   num_devices=N_CORES)

    xq = nc.dram_tensor("xq", (NMC, 128, KT, MC), BF16, kind="ExternalInput").ap()
    xk = nc.dram_tensor("xk", (NMC, 128, KT, MC), BF16, kind="ExternalInput").ap()
    xv = nc.dram_tensor("xv", (NMC, 128, KT, MC), BF16, kind="ExternalInput").ap()
    wq = nc.dram_tensor("wq", (128, KT, CG), BF16, kind="ExternalInput").ap()
    wk = nc.dram_tensor("wk", (128, KT, CG), BF16, kind="ExternalInput").ap()
    wv = nc.dram_tensor("wv", (128, KT, CG), BF16, kind="ExternalInput").ap()
    bq = nc.dram_tensor("bq", (128, MT), F32, kind="ExternalInput").ap()
    bk = nc.dram_tensor("bk", (128, MT), F32, kind="ExternalInput").ap()
    bv = nc.dram_tensor("bv", (CG,), F32, kind="ExternalInput").ap()
    wo = nc.dram_tensor("wo", (128, MT, C), BF16, kind="ExternalInput").ap()
    yp = nc.dram_tensor("yp", (T, C), BF16, kind="ExternalOutput").ap()

    with tile.TileContext(nc) as tc:
        _emit(tc, xq, xk, xv, wq, wk, wv, bq, bk, bv, wo, yp)
    nc.compile()
    return nc


def _emit(tc, xq, xk, xv, wq, wk, wv, bq, bk, bv, wo, yp):
    nc = tc.nc

    from contextlib import ExitStack
    with ExitStack() as ctx:
        consts = ctx.enter_context(tc.tile_pool(name="consts", bufs=1))
        xs_pool = ctx.enter_context(tc.tile_pool(name="xs", bufs=18))
        big = ctx.enter_context(tc.tile_pool(name="big", bufs=1))
        e_pool = ctx.enter_context(tc.tile_pool(name="e", bufs=20))
        ev_pool = ctx.enter_context(tc.tile_pool(name="ev", bufs=3))
        nrm_pool = ctx.enter_context(tc.tile_pool(name="nrm", bufs=4))
        pp = ctx.enter_context(tc.tile_pool(name="pp", bufs=2, space="PSUM"))
        pv_ps = ctx.enter_context(tc.tile_pool(name="pvps", bufs=2, space="PSUM"))
        sa_ps = ctx.enter_context(tc.tile_pool(name="saps", bufs=1, space="PSUM"))
        sb_ps = ctx.enter_context(tc.tile_pool(name="sbps", bufs=1, space="PSUM"))

        qTc = [big.tile([128, MT, MC], BF16, name=f"qTc{i}", tag=f"qTc{i}")
               for i in range(NMC)]
        kTc = [big.tile([128, MT, MC], BF16, name=f"kTc{i}", tag=f"kTc{i}")
               for i in range(NMC)]
        vc = [big.tile([128, MC // 128, HG * VS], BF16, name=f"vc{i}",
                       tag=f"vc{i}") for i in range(NMC)]
        # O packed head-pair-major: partitions 0-63 head 2hp, 64-127 head
        # 2hp+1, so the out-projection contracts over all 128 partitions.
        oc = [big.tile([128, MT, QC], BF16, name=f"oc{i}", tag=f"oc{i}")
              for i in range(NQC)]

        wq_s = consts.tile([128, KT, CG], BF16, tag="wq")
        wk_s = consts.tile([128, KT, CG], BF16, tag="wk")
        wv_s = consts.tile([128, KT, CG], BF16, tag="wv")
        wo_s = consts.tile([128, MT, C], BF16, tag="wo")
        bq_s = consts.tile([128, MT, 1], F32, tag="bq")
        bk_s = consts.tile([128, MT, 1], F32, tag="bk")
        bv_bc = consts.tile([128, CG], F32, tag="bv")

        def load_x(src, name, dtype=BF16):
            """Load one x chunk as two half tiles (kt 0-3, 4-7) so the
            first projection matmuls can start after half the transfer."""
            halves = []
            for h in range(2):
                x_t = xs_pool.tile([128, KT // 2, MC], dtype, tag="xs",
                                   name=f"{name}_{h}")
                nc.sync.dma_start(out=x_t, in_=src[:, bass.ts(h, KT // 2), :])
                halves.append(x_t)
            return halves

        def emit_qk_proj(x_h, w_s, b_s, dst, mts=(0, 1)):
            for mt in mts:
                ps = pp.tile([128, 512], F32, tag="pp")
                for kt in range(KT):
                    nc.tensor.matmul(
                        ps[:, :MC],
                        w_s[:, kt, bass.ts(mt, 128)],
                        x_h[kt // 4][:, kt % 4, :],
                        start=(kt == 0), stop=(kt == KT - 1))
                nc.vector.tensor_scalar_add(
                    dst[:, mt, :], ps[:, :MC], b_s[:, mt, :])

        def emit_v_proj(x_h, mc, subs=(0, 1, 2, 3)):
            v4 = vc[mc].rearrange("p t (h c) -> p t h c", h=HG)
            if 0 in subs:
                nc.vector.memset(v4[:, :, :, D:VS], 1.0)
            for sub in subs:
                ps = pp.tile([128, 512], F32, tag="pp")
                for kt in range(KT):
                    nc.tensor.matmul(
                        ps[:, :CG],
                        x_h[kt // 4][:, kt % 4, bass.ts(sub, 128)],
                        wv_s[:, kt, :],
                        start=(kt == 0), stop=(kt == KT - 1))
                nc.vector.tensor_add(
                    v4[:, sub, :, 0:D],
                    ps[:, :CG].rearrange("p (h c) -> p h c", h=HG),
                    bv_bc.rearrange("p (h c) -> p h c", h=HG))

        def emit_d_unit(qc, tl, cc):
            trows_out = bass.ts(qc * (QC // 128) + tl, 128)
            ps = pp.tile([128, 512], F32, tag="pp")
            for hp in range(MT):
                nc.tensor.matmul(
                    ps,
                    oc[qc][:, hp, bass.ts(tl, 128)],
                    wo_s[:, hp, bass.ts(cc, 512)],
                    start=(hp == 0), stop=(hp == MT - 1))
            ev = ev_pool.tile([128, 512], BF16, tag="ev")
            # In the post-ACT tail, split the PSUM evacuations between the
            # (now idle) ScalarE and the DVE so they don't serialize.
            if qc == NQC - 1 and (2 * tl + cc) % 2 == 0:
                nc.scalar.copy(ev, ps)
            else:
                nc.vector.tensor_copy(ev, ps)
            nc.sync.dma_start(out=yp[trows_out, bass.ts(cc, 512)], in_=ev)

        e_store = {}

        def unit_scores(qc, hp):
            """Scores + exp for one (q-chunk, head-pair): 8 ticks of 2 tk."""
            lst = e_store[(qc, hp)] = []
            for tk in range(TT):
                pool = sa_ps if tk % 2 == 0 else sb_ps
                ps = pool.tile([128, 2, 512], F32,
                               tag="sa" if tk % 2 == 0 else "sb")
                for h01 in range(2):
                    pb = h01 * D
                    nc.tensor.matmul(
                        ps[:, h01, :],
                        kTc[tk // 4][pb:pb + D, hp, bass.ts(tk % 4, 128)],
                        qTc[qc][pb:pb + D, hp, :],
                        start=True, stop=True)
                e_t = e_pool.tile([128, 2, 512], BF16, tag="e")
                nc.scalar.activation(e_t, ps, AF.Exp, scale=SCALE)
                lst.append(e_t)
                if tk % 2 == 1:
                    yield

        def unit_pv(qc, hp):
            """PV accumulation + normalization, trailing unit_scores by one
            full unit so every exp it reads is long finished."""
            lst_get = lambda tk: e_store[(qc, hp)][tk]
            po = [pv_ps.tile([128, 512], F32, tag="pv",
                             name=f"po{qc}_{hp}_{i}") for i in range(2)]
            for tk in range(TT):
                for h01 in range(2):
                    nc.tensor.matmul(
                        po[h01][0:VS, :],
                        vc[tk // 4][:, tk % 4,
                                    bass.ds((2 * hp + h01) * VS, VS)],
                        lst_get(tk)[:, h01, :],
                        start=(tk == 0), stop=(tk == TT - 1))
                if tk % 4 == 3 and tk != TT - 1:
                    yield
            last = (qc, hp) == (NQC - 1, MT - 1)
            if last:
                # Nothing needs these PSUM banks afterwards: normalize
                # straight from po with the shortest possible chain.
                for h01 in range(2):
                    t = nrm_pool.tile([128, 512], BF16, tag="ou")
                    if h01 == 0:
                        nc.scalar.copy(t[D:D + 1, :], po[h01][D:D + 1, :])
                    else:
                        nc.vector.tensor_copy(t[D:D + 1, :],
                                              po[h01][D:D + 1, :])
                    rb = pp.tile([128, 512], F32, tag="pp")
                    nc.tensor.matmul(rb[0:64, :], ones_b[D:D + 1, :],
                                     t[D:D + 1, :], start=True, stop=True)
                    rec = nrm_pool.tile([64, 512], F32, tag="rec")
                    nc.vector.reciprocal_approx_fast(rec, rb[0:64, :])
                    nc.vector.tensor_mul(
                        oc[qc][bass.ts(h01, 64), hp, :],
                        po[h01][0:D, :], rec)
                del e_store[(qc, hp)]
                yield
                return
            # Evacuate po (O cols + den col) to SBUF right away so the two
            # PSUM banks free in ~1.4us for the next unit's PV; the actual
            # normalization then runs off the critical path.
            ou = []
            for h01 in range(2):
                t = nrm_pool.tile([128, 512], BF16, tag="ou")
                nc.vector.tensor_copy(t[0:VS, :], po[h01][0:VS, :])
                ou.append(t)
            for h01 in range(2):
                rb = pp.tile([128, 512], F32, tag="pp")
                nc.tensor.matmul(rb[0:64, :], ones_b[D:D + 1, :],
                                 ou[h01][D:D + 1, :], start=True, stop=True)
                rec = nrm_pool.tile([64, 512], F32, tag="rec")
                nc.vector.reciprocal_approx_fast(rec, rb[0:64, :])
                nc.vector.tensor_mul(
                    oc[qc][bass.ts(h01, 64), hp, :],
                    ou[h01][0:D, :], rec)
            del e_store[(qc, hp)]
            yield

        # ---- DMA issue (priority order) -------------------------------
        nc.sync.dma_start(out=wq_s, in_=wq)
        nc.sync.dma_start(out=bq_s, in_=bq.unsqueeze(2))
        xq_t = [None] * NMC
        xk_t = [None] * NMC
        xv_t = [None] * NMC
        xq_t[0] = load_x(xq[0], "xq0")
        nc.sync.dma_start(out=wk_s, in_=wk)
        nc.sync.dma_start(out=bk_s, in_=bk.unsqueeze(2))
        xk_t[0] = load_x(xk[0], "xk0")
        nc.sync.dma_start(out=wv_s, in_=wv)
        nc.sync.dma_start(
            out=bv_bc,
            in_=bass.AP(tensor=bv.tensor, offset=bv.offset,
                        ap=[[0, 128]] + list(bv.ap)))
        xv_t[0] = load_x(xv[0], "xv0")
        for mc in range(1, NMC):
            xk_t[mc] = load_x(xk[mc], f"xk{mc}")
        for mc in range(1, NMC):
            xv_t[mc] = load_x(xv[mc], f"xv{mc}")
        xq_t[1] = load_x(xq[1], "xq1")
        nc.sync.dma_start(out=wo_s, in_=wo)

        # touch Exp once so the ACT table loads during the DMA-bound head
        warm = consts.tile([128, 1], F32, tag="warm")
        nc.vector.memset(warm[0:1, :], 0.0)
        nc.scalar.activation(warm[0:1, :], warm[0:1, :], AF.Exp)
        ones_b = consts.tile([128, 64], BF16, tag="onesb")
        nc.vector.memset(ones_b[D:D + 1, :], 1.0)

        # ---- slot pipeline --------------------------------------------
        # Each slot = one 4-tk block of the scores+exp stream (~4.4us of
        # ACT).  Fills (projection chains, out-proj units, late DMA
        # issues) are emitted AFTER the slot's scores so the exp stream
        # stays at the head of the PE queue; a chain consumed by the
        # scores of slot j is emitted in a slot < j (PE is strict FIFO —
        # emitting it later would deadlock).
        emit_qk_proj(xq_t[0], wq_s, bq_s, qTc[0])
        emit_qk_proj(xk_t[0], wk_s, bk_s, kTc[0])

        def load_late(mc):
            def f():
                xq_t[mc] = load_x(xq[mc], f"xq{mc}")
            return f

        def qk(xl, w, b, dst, mts):
            return lambda: emit_qk_proj(xl, w, b, dst, mts=mts)

        def vp(mc, subs):
            return lambda: emit_v_proj(xv_t[mc], mc, subs=subs)

        def qp(mc, mts):
            return lambda: emit_qk_proj(xq_t[mc], wq_s, bq_s, qTc[mc],
                                        mts=mts)

        # Fills at half-chain granularity, spread so no slot's PE load
        # exceeds the ~4.4us of ACT time per slot (deadlines in comments:
        # scores blk j needs K_j by slot j-1; pv(0) blk j needs V_j by
        # slot j+3; unit (qc,hp) needs Q_qc by slot 4*(2qc+hp)-1).
        forced = {
            0: [qk(xk_t[1], wk_s, bk_s, kTc[1], (0, 1))],
            1: [qk(xk_t[2], wk_s, bk_s, kTc[2], (0, 1))],
            2: [qk(xk_t[3], wk_s, bk_s, kTc[3], (0, 1)), vp(0, (0, 1))],
            3: [vp(0, (2, 3)), vp(1, (0, 1)), load_late(2)],
            4: [vp(1, (2, 3)), vp(2, (0, 1)), load_late(3)],
            5: [vp(2, (2, 3)), vp(3, (0, 1))],
            6: [vp(3, (2, 3)), qp(1, (0,))],
            7: [qp(1, (1,))],
            8: [qp(2, (0,))],
            9: [qp(2, (1,))],
            10: [qp(3, (0,))],
            11: [qp(3, (1,))],
        }
        fills = deque()

        units = [(qc, hp) for qc in range(NQC) for hp in range(MT)]
        sg = [unit_scores(*u) for u in units]
        pg = [unit_pv(*u) for u in units]
        d_queue = []
        nslots = 4 * len(units)
        nticks = 8 * len(units)
        # Scores advance in 2-tk ticks, half a block ahead of the slot
        # machinery, so the ACT stream has a ~2us exp buffer that absorbs
        # per-slot PE bursts (fills, pv, d-units) without starving.
        st = [0]

        def adv_scores():
            if st[0] < nticks:
                next(sg[st[0] // 8], None)
                st[0] += 1

        # pv stream trails scores by 4 slots at the head (relaxes the K/V
        # chain deadlines there) and catches up mid-stream (double-advance
        # slots, placed so both advances stay inside one unit) to reach
        # lag 1 at the end, minimizing the post-ACT tail.
        CATCHUP = {14, 19, 24}
        pv_pos = 0
        adv_scores()                       # tick 0 (tks 0-1)
        for slot in range(nslots + 1):
            adv_scores()
            for f in forced.pop(slot, ()):
                f()
            if slot >= 8:
                for _ in range(2):
                    if d_queue:
                        emit_d_unit(*d_queue.pop(0))
                    elif fills:
                        fills.popleft()()
                        break
            adv_scores()
            n_adv = 0 if slot < 4 else (2 if slot in CATCHUP else 1)
            for _ in range(n_adv):
                if pv_pos < nslots:
                    next(pg[pv_pos // 4], None)
                    if pv_pos % 4 == 3:
                        qc, hp = units[pv_pos // 4]
                        if hp == 1:
                            d_queue += [(qc, tl, cc)
                                        for tl in range(4) for cc in range(2)]
                    pv_pos += 1
        for u in d_queue:
            emit_d_unit(*u)


_NC_CACHE = None


def _get_program():
    global _NC_CACHE
    if _NC_CACHE is None:
        _NC_CACHE = build_mha_program()
    return _NC_CACHE


def make_in_maps(query, key, value, Wq, bq, Wk, bk, Wv, bv, Wo):
    q = np.asarray(query, np.float32).reshape(B, T, C)
    k = np.asarray(key, np.float32).reshape(B, T, C)
    v = np.asarray(value, np.float32).reshape(B, T, C)
    # [mc][p][kt][m] = x[b][mc*512 + m][kt*128 + p] : exact SBUF tile layout,
    # contiguous per chunk so each chunk DMA is one fat 2D descriptor.
    xT = {n: [np.ascontiguousarray(
                  a[b].T.reshape(KT, 128, NMC, MC).transpose(2, 1, 0, 3))
              .astype(ml_dtypes.bfloat16)
              for b in range(B)]
          for n, a in (("q", q), ("k", k), ("v", v))}
    in_maps = []
    for c in range(N_CORES):
        b, g = divmod(c, GROUPS)
        sl = slice(g * CG, (g + 1) * CG)
        wqg = np.asarray(Wq, np.float32)[:, sl].reshape(KT, 128, CG)
        wkg = np.asarray(Wk, np.float32)[:, sl].reshape(KT, 128, CG)
        wvg = np.asarray(Wv, np.float32)[:, sl].reshape(KT, 128, CG)
        wog = np.asarray(Wo, np.float32)[sl, :].reshape(MT, 128, C)
        in_maps.append({
            "xq": xT["q"][b], "xk": xT["k"][b], "xv": xT["v"][b],
            "wq": np.ascontiguousarray(wqg.transpose(1, 0, 2)).astype(ml_dtypes.bfloat16),
            "wk": np.ascontiguousarray(wkg.transpose(1, 0, 2)).astype(ml_dtypes.bfloat16),
            "wv": np.ascontiguousarray(wvg.transpose(1, 0, 2)).astype(ml_dtypes.bfloat16),
            "bq": np.ascontiguousarray(
                np.asarray(bq, np.float32)[sl].reshape(MT, 128).T),
            "bk": np.ascontiguousarray(
                np.asarray(bk, np.float32)[sl].reshape(MT, 128).T),
            "bv": np.ascontiguousarray(np.asarray(bv, np.float32)[sl]),
            "wo": np.ascontiguousarray(wog.transpose(1, 0, 2)).astype(ml_dtypes.bfloat16),
        })
    return in_maps


def assemble_output(results, bo):
    y = np.zeros((B, T, C), np.float32)
    for c, res in enumerate(results):
        y[c // GROUPS] += np.asarray(res["yp"], np.float32)
    y += np.asarray(bo, np.float32)
    return y


def kernel(query, key, value, Wq, bq, Wk, bk, Wv, bv, Wo, bo):
    nc = _get_program()
    in_maps = make_in_maps(query, key, value, Wq, bq, Wk, bk, Wv, bv, Wo)
    res = run_bass_kernel_spmd(nc, in_maps, list(range(N_CORES)))
    return assemble_output(res.results, bo)
